# revision 8
# baseline (speedup 1.0000x reference)
"""2-layer GAT (GATConv x2, PyG-style self-loops) on 8 Trainium2 NeuronCores.

Strategy (graph parallel, nodes+incident-edges partitioned by destination):
- Nodes sharded across 8 cores (12500 each, padded to 12544). Each core
  projects its shard (x @ W1aug with the attention vectors folded in),
  writes per-node table rows [al_src | feat] to DRAM, and AllGathers the
  table so every core holds all source rows (halo).
- The per-edge source-row gather is the bottleneck.  Rows are laid out
  band-major: 4 "bands" of positions across all cores, each band section
  <= 25600 rows so a row index fits int16.  Per 128-destination tile,
  incident edges sit in slot columns grouped by source band; each
  (tile, band) range is fetched by batched InstDMAGatherAnt instructions
  (<=896 indices each, one SWDGE queue per band so descriptor generation
  runs on all four Q7 core pairs concurrently).  This measures ~2.4 ns
  per gathered row vs ~8.6 ns/row for the one-column-per-instruction
  indirect-DMA path.
- Attention softmax + weighted aggregation run as strided DVE/ACT ops over
  the gathered block; max-subtraction is skipped (logits are O(1), and
  exp(e-m)/sum == exp(e)/sum analytically).  Slot padding points at
  per-band dummy rows whose al_src is -60000 so exp() contributes 0.
- AllGathers are chunked per band so layer gathers overlap the collective.
- Weights are tiny and replicated; a_src/a_dst are folded into the
  projections on the host: W_aug = [W @ Asrc | W | W @ Adst].
"""

import numpy as np

# Problem constants (hardcoded per spec)
N = 100000
E = 1600000
F_IN = 512
HID = 8
HEADS = 8
F_HID = HID * HEADS  # 64
NUM_CLASSES = 40
NEG_SLOPE = 0.2
CORES = 8
P = 128
PITCH = 128          # fp16 elements per table row (256B DMA stride)
T1W = HEADS + F_HID  # 72: [al_s(8) | h(64)]
T2W = 1 + NUM_CLASSES  # 41: [al_s2(1) | g(40)]
NBANDS = 4
BSIZES = [3200, 3200, 3200, 2944]     # positions per band (per core)
MAX_NI = 448                          # 2 per 64-desc SWDGE ring: pipelined
BIG_NEG = -60000.0

_PROGRAM_CACHE = {}


def _split_shards(n, cores):
    base = n // cores
    rem = n % cores
    sizes = [base + (1 if r < rem else 0) for r in range(cores)]
    offs = np.concatenate([[0], np.cumsum(sizes)])
    return sizes, offs


def _preprocess_graph(edge_index, n=N, cores=CORES):
    src = edge_index[0].astype(np.int64)
    dst = edge_index[1].astype(np.int64)

    sizes, offs = _split_shards(n, cores)
    shard_pad = int(np.ceil((max(sizes) + 1) / P) * P)  # 12544
    tiles = shard_pad // P                              # 98
    assert sum(BSIZES) == shard_pad

    band_pos_off = np.concatenate([[0], np.cumsum(BSIZES)])   # per-core pos
    band_tab_off = np.concatenate([[0], np.cumsum([8 * b for b in BSIZES])])
    caps = [b - 1 for b in BSIZES]  # last row of each band = dummy

    deg = np.bincount(dst, minlength=n)
    node_core = np.searchsorted(offs[1:], np.arange(n), side="right")

    # pass 1: per-core degree sort -> positions (band assignment)
    node_pos = np.full(n, -1, dtype=np.int64)
    real_pos_template = np.concatenate(
        [np.arange(band_pos_off[b], band_pos_off[b] + caps[b])
         for b in range(NBANDS)])
    for r in range(cores):
        own = np.arange(offs[r], offs[r + 1])
        order = own[np.argsort(-deg[own], kind="stable")]
        node_pos[order] = real_pos_template[:len(order)]

    pos_band = np.searchsorted(band_pos_off[1:], np.arange(shard_pad),
                               side="right")
    src_band = pos_band[node_pos[src]]

    # per-node per-band in-degree (for clustering + slot packing)
    degb = np.zeros((n, NBANDS), dtype=np.int64)
    np.add.at(degb, (dst, src_band), 1)

    # pass 2: within each band, cluster nodes into tiles by band-profile
    # (bucketed lexsort) -- band membership (and hence src_band) unchanged.
    for r in range(cores):
        own_mask_pos = np.full(shard_pad, -1, dtype=np.int64)
        own = np.arange(offs[r], offs[r + 1])
        own_mask_pos[node_pos[own]] = own
        for b in range(NBANDS):
            pos_lo = band_pos_off[b]
            nodes_b = own_mask_pos[pos_lo:pos_lo + caps[b]]
            nodes_b = nodes_b[nodes_b >= 0]
            if len(nodes_b) == 0:
                continue
            d = degb[nodes_b]
            bb = d // 4
            key = np.lexsort((-deg[nodes_b], -bb[:, 3], -bb[:, 2],
                              -bb[:, 1], -bb[:, 0]))
            node_pos[nodes_b[key]] = pos_lo + np.arange(len(nodes_b))

    table_row = np.zeros(n, dtype=np.int64)
    pb = pos_band[node_pos]
    table_row = (band_tab_off[pb] +
                 node_core * np.array(BSIZES)[pb] +
                 (node_pos - band_pos_off[pb]))
    src_band = pos_band[node_pos[src]]  # unchanged by pass 2, recompute anyway

    # per-tile per-band slot counts, shared across cores
    Dq = np.zeros((tiles, NBANDS), dtype=np.int64)
    for r in range(cores):
        m = (dst >= offs[r]) & (dst < offs[r + 1])
        lpos = node_pos[dst[m]]
        sb = src_band[m]
        cnt = np.zeros((shard_pad, NBANDS), dtype=np.int64)
        np.add.at(cnt, (lpos, sb), 1)
        Dq = np.maximum(Dq, cnt.reshape(tiles, P, NBANDS).max(axis=1))
    # every tile keeps >=1 band-0 slot (pad-dst den>0 safety)
    Dq[:, 0] = np.maximum(Dq[:, 0], 1)

    SL = Dq.sum(axis=1)                 # slots per tile (excl. self-loop)
    tile_col_off = np.concatenate([[0], np.cumsum(SL)])
    S_cols = int(SL.sum())

    # fill per-core idx16 arrays [P, S_cols]
    dummy_local = [BSIZES[b] - 1 for b in range(NBANDS)]  # core 0's dummy
    band_col_off = np.concatenate(
        [np.zeros((tiles, 1), np.int64), np.cumsum(Dq, axis=1)], axis=1)
    idx_arrays = []
    for r in range(cores):
        idx = np.zeros((P, S_cols), dtype=np.int16)
        for b in range(NBANDS):
            cols_b = []
            for t in range(tiles):
                c0 = tile_col_off[t] + band_col_off[t][b]
                cols_b.append(np.arange(c0, c0 + Dq[t][b]))
            cols_b = np.concatenate(cols_b)
            idx[:, cols_b] = dummy_local[b]
        m = (dst >= offs[r]) & (dst < offs[r + 1])
        e_src = src[m]
        lpos = node_pos[dst[m]]
        sb = src_band[m]
        o = np.lexsort((sb, lpos))
        e_src, lpos, sb = e_src[o], lpos[o], sb[o]
        key = lpos * NBANDS + sb
        grp_start = np.searchsorted(key, np.arange(shard_pad * NBANDS),
                                    side="left")
        slot = np.arange(len(lpos)) - grp_start[key]
        t_of = lpos // P
        part = lpos % P
        col = (tile_col_off[t_of] + band_col_off[t_of, sb] + slot)
        loc = (table_row[e_src] - band_tab_off[sb]).astype(np.int16)
        idx[part, col] = loc
        # pad dst rows: give band-0 slot 0 a real row (local 0)
        npos = np.zeros(shard_pad, dtype=bool)
        own = np.arange(offs[r], offs[r + 1])
        npos[node_pos[own]] = True
        pl = np.nonzero(~npos)[0]
        c0 = tile_col_off[pl // P] + band_col_off[pl // P, 0]
        idx[pl % P, c0] = 0
        idx_arrays.append(idx)

    # gather-instruction metas + int16 streams (wrapped + 8x replicated)
    metas = []  # (tile, band, col_rel, ncols, stream_off16, NI)
    blocks = []
    w_off = 0
    for t in range(tiles):
        chunk_list = []
        for b in range(NBANDS):
            nb = int(Dq[t][b])
            ca = 0
            while ca < nb:
                w = min(MAX_NI // P, nb - ca)
                chunk_list.append((b, ca, w))
                ca += w
        # round-robin across bands so consecutive instrs hit different queues
        chunk_list.sort(key=lambda x: (x[1] // max(x[2], 1), x[0]))
        for b, ca, w in chunk_list:
            NI = w * P
            col0 = int(tile_col_off[t] + band_col_off[t][b] + ca)
            metas.append((t, b, int(band_col_off[t][b] + ca), w, w_off, NI))
            blocks.append((col0, w))
            w_off += w * 8

    def build_stream(idx):
        stream = np.zeros((P, w_off), dtype=np.int16)
        o = 0
        for col0, w in blocks:
            NI = w * P
            flat = idx[:, col0:col0 + w].T.ravel()
            wr = np.zeros((16, w * 8), dtype=np.int16)
            wr[np.arange(NI) % 16, np.arange(NI) // 16] = flat
            stream[:, o:o + w * 8] = np.tile(wr, (8, 1))
            o += w * 8
        return stream

    streams = [build_stream(idx_arrays[r]) for r in range(cores)]

    # pad region: positions never assigned to a real node (incl. band dummies)
    assigned = np.zeros(shard_pad, dtype=bool)
    assigned[node_pos[np.arange(offs[0], offs[1])]] = True  # same per core
    neg_rows = np.nonzero(~assigned)[0]

    return {
        "streams": streams, "metas": metas, "Dq": Dq, "SL": SL,
        "tile_col_off": tile_col_off, "band_col_off": band_col_off,
        "node_pos": node_pos, "sizes": sizes, "offs": offs,
        "shard_pad": shard_pad, "tiles": tiles, "S_cols": S_cols,
        "W16": w_off, "neg_rows": neg_rows,
        "band_pos_off": band_pos_off, "band_tab_off": band_tab_off,
    }


def _build_program(meta):
    from concourse import mybir, bacc
    import concourse.tile as tile
    import concourse.ap_utils as ap_utils
    from concourse.masks import make_identity

    dt = mybir.dt
    SH = meta["shard_pad"]
    TILES = meta["tiles"]
    SL = meta["SL"]
    W16 = meta["W16"]
    NROWS = SH * CORES
    W1C = F_IN // P
    band_pos_off = meta["band_pos_off"]
    band_tab_off = meta["band_tab_off"]
    neg_rows = meta["neg_rows"]
    metas = meta["metas"]

    nc = bacc.Bacc("TRN2", target_bir_lowering=False, debug=False,
                   num_devices=CORES, num_swdge_queues=4)
    xT = nc.dram_tensor("xT", [F_IN, SH], dt.float16, kind="ExternalInput")
    w1aug = nc.dram_tensor("w1aug", [F_IN, 80], dt.float16, kind="ExternalInput")
    w2aug = nc.dram_tensor("w2aug", [F_HID, 42], dt.float16, kind="ExternalInput")
    b1rep = nc.dram_tensor("b1rep", [P, F_HID], dt.float32, kind="ExternalInput")
    b2rep = nc.dram_tensor("b2rep", [P, NUM_CLASSES], dt.float32, kind="ExternalInput")
    idx_in = nc.dram_tensor("idx16", [P, W16], dt.int16, kind="ExternalInput")
    out = nc.dram_tensor("out", [SH, NUM_CLASSES], dt.float32, kind="ExternalOutput")

    AF = mybir.ActivationFunctionType
    OP = mybir.AluOpType
    AX = mybir.AxisListType

    def raw_gather(G_ap, in_ap, idxs_ap, NI, elem, queue):
        g = nc.gpsimd
        assert idxs_ap.dtype == dt.int16
        assert ap_utils.ap_is_contiguous(in_ap.ap[1:])
        assert ap_utils.ap_is_contiguous(G_ap.ap[1:])
        assert ap_utils.ap_is_contiguous(idxs_ap.ap[1:])
        assert in_ap.ap[0][0] == PITCH
        stride_bytes_256 = (PITCH * 2) // 256
        _in_ap = g.lower_ap_dma(in_ap, for_custom_bir_dma=True)
        _idxs_ap = g.lower_ap(idxs_ap)
        _out_ap = g.lower_ap(G_ap)
        return g.add_instruction(
            mybir.InstDMAGatherAnt(
                name=nc.get_next_instruction_name(),
                ins=[*_in_ap, _idxs_ap, g.lower_val_access(g.to_reg(NI))],
                outs=[_out_ap],
                transpose=False, num_idxs=NI, elem_size=elem,
                stride_bytes_256=stride_bytes_256, gen_mode=0,
                single_packet=True, queue_num=queue,
                sbuf_tokens_per_rank=0, sbuf_free_dim_per_rank=0,
                sbuf_free_dim_pad_per_rank=0, sbuf_byte_offset=0))

    with tile.TileContext(nc) as tc:
        with (
            tc.tile_pool(name="const", bufs=1) as cpool,
            tc.tile_pool(name="resident", bufs=1) as rpool,
            tc.tile_pool(name="work", bufs=3) as wpool,
            tc.tile_pool(name="gbuf", bufs=3) as gpool,
            tc.tile_pool(name="psum", bufs=2, space="PSUM") as ppool,
            tc.tile_pool(name="dram", bufs=1, space="DRAM") as dpool,
        ):
            # ---- constants / residents ----
            w1_t = cpool.tile([P, W1C * 80], dt.float16)
            for c in range(W1C):
                nc.sync.dma_start(out=w1_t[:, c * 80:(c + 1) * 80],
                                  in_=w1aug[c * P:(c + 1) * P, :])
            w2_t = cpool.tile([F_HID, 42], dt.float16)
            nc.sync.dma_start(out=w2_t[:], in_=w2aug[:, :])
            b1_t = cpool.tile([P, F_HID], dt.float32)
            nc.sync.dma_start(out=b1_t[:], in_=b1rep[:, :])
            b2_t = cpool.tile([P, NUM_CLASSES], dt.float32)
            nc.sync.dma_start(out=b2_t[:], in_=b2rep[:, :])
            ident = cpool.tile([P, P], dt.float32)
            make_identity(nc, ident[:])
            idx_t = rpool.tile([P, W16], dt.int16)
            nc.sync.dma_start(out=idx_t[:], in_=idx_in[:, :])
            ald1 = rpool.tile([P, TILES * HEADS], dt.float32)
            ald2 = rpool.tile([P, TILES], dt.float32)

            t1_shard = dpool.tile([SH, PITCH], dt.float16)
            t1_full = dpool.tile([NROWS, PITCH], dt.float16)
            t2_shard = dpool.tile([SH, PITCH], dt.float16)
            t2_full = dpool.tile([NROWS, PITCH], dt.float16)

            # ---- phase A: h1 = x @ W1aug per 128-node tile ----
            for t in range(TILES):
                ps = ppool.tile([P, 80], dt.float32, tag="psA")
                lhs = wpool.tile([P, W1C * P], dt.float16, tag="xT")
                nc.sync.dma_start(
                    out=lhs[:].rearrange("p (c n) -> p c n", n=P),
                    in_=xT[:, t * P:(t + 1) * P]
                        .rearrange("(c p) n -> p c n", p=P))
                for c in range(W1C):
                    nc.tensor.matmul(
                        out=ps[:], lhsT=lhs[:, c * P:(c + 1) * P],
                        rhs=w1_t[:, c * 80:(c + 1) * 80],
                        start=(c == 0), stop=(c == W1C - 1))
                row = wpool.tile([P, T1W], dt.float16, tag="t1row")
                nc.scalar.copy(row[:], ps[:, 0:T1W])
                nc.sync.dma_start(out=t1_shard[t * P:(t + 1) * P, 0:T1W],
                                  in_=row[:])
                nc.vector.tensor_copy(ald1[:, t * HEADS:(t + 1) * HEADS],
                                      ps[:, T1W:80])
            # dummy/pad rows -> BIG_NEG so their slots contribute 0
            dummy = wpool.tile([P, PITCH], dt.float16, tag="dummy")
            nc.vector.memset(dummy[:], BIG_NEG)
            runs = []  # contiguous runs of neg_rows
            for rrow in neg_rows:
                if runs and runs[-1][1] == rrow:
                    runs[-1][1] = rrow + 1
                else:
                    runs.append([rrow, rrow + 1])
            for a, b in runs:
                nc.sync.dma_start(out=t1_shard[a:b, :], in_=dummy[:b - a, :])

            # ---- AllGather layer-1 table, chunked per band ----
            for b in range(NBANDS):
                po, bs = int(band_pos_off[b]), BSIZES[b]
                to = int(band_tab_off[b])
                nc.gpsimd.collective_compute(
                    "AllGather", OP.bypass,
                    replica_groups=[list(range(CORES))],
                    ins=[t1_shard[po:po + bs, :].opt()],
                    outs=[t1_full[to:to + 8 * bs, :].opt()])

            # ---- phase C1: layer-1 aggregation + layer-2 projection ----
            mi = 0
            for t in range(TILES):
                NSL = int(SL[t]) + 1  # + self-loop column (last)
                G = gpool.tile([P, NSL * T1W], dt.float16, tag="G1", bufs=5)
                while mi < len(metas) and metas[mi][0] == t:
                    _, b, crel, w, so, NI = metas[mi]
                    to = int(band_tab_off[b])
                    bs8 = 8 * BSIZES[b]
                    raw_gather(
                        G[:, crel * T1W:(crel + w) * T1W]
                            .rearrange("p (c e) -> p c e", e=T1W),
                        t1_full[to:to + bs8, 0:T1W],
                        idx_t[:, so:so + w * 8], NI, T1W, b)
                    mi += 1
                nc.sync.dma_start(
                    out=G[:, (NSL - 1) * T1W:NSL * T1W],
                    in_=t1_shard[t * P:(t + 1) * P, 0:T1W])
                plog = wpool.tile([P, NSL * HEADS], dt.float32, tag="plog")
                G_al = G[:].rearrange("p (d w) -> p d w", w=T1W)[:, :, 0:HEADS]
                ald_b = ald1[:, t * HEADS:(t + 1) * HEADS].unsqueeze(1) \
                    .broadcast_to([P, NSL, HEADS])
                nc.vector.tensor_tensor(
                    out=plog[:].rearrange("p (d w) -> p d w", w=HEADS),
                    in0=G_al, in1=ald_b, op=OP.add)
                nc.vector.scalar_tensor_tensor(
                    out=plog[:], in0=plog[:], scalar=NEG_SLOPE, in1=plog[:],
                    op0=OP.mult, op1=OP.max)
                plog16 = wpool.tile([P, NSL * HEADS], dt.float16, tag="plog16")
                nc.scalar.activation(plog16[:], plog[:], AF.Exp)
                den = wpool.tile([P, HEADS], dt.float32, tag="den")
                nc.vector.tensor_reduce(
                    out=den[:],
                    in_=plog16[:].rearrange("p (d w) -> p w d", w=HEADS),
                    axis=AX.X, op=OP.add)
                rec = wpool.tile([P, HEADS], dt.float32, tag="rec")
                nc.vector.reciprocal(rec[:], den[:])
                Gp = wpool.tile([P, NSL * F_HID], dt.float16, tag="Gp")
                G_h = G[:].rearrange("p (d w) -> p d w", w=T1W)[:, :, HEADS:T1W] \
                    .rearrange("p d (h f) -> p d h f", f=HID)
                p_b = plog16[:].rearrange("p (d h) -> p d h", h=HEADS) \
                    .unsqueeze(3).broadcast_to([P, NSL, HEADS, HID])
                nc.vector.tensor_tensor(
                    out=Gp[:].rearrange("p (d h f) -> p d h f", h=HEADS, f=HID),
                    in0=G_h, in1=p_b, op=OP.mult)
                acc = wpool.tile([P, F_HID], dt.float32, tag="acc")
                nc.vector.tensor_reduce(
                    out=acc[:],
                    in_=Gp[:].rearrange("p (d w) -> p w d", w=F_HID),
                    axis=AX.X, op=OP.add)
                h2 = wpool.tile([P, F_HID], dt.float32, tag="h2")
                rec_b = rec[:].unsqueeze(2).broadcast_to([P, HEADS, HID])
                nc.vector.tensor_tensor(
                    out=h2[:].rearrange("p (h f) -> p h f", f=HID),
                    in0=acc[:].rearrange("p (h f) -> p h f", f=HID),
                    in1=rec_b, op=OP.mult)
                nc.vector.tensor_tensor(out=h2[:], in0=h2[:], in1=b1_t[:], op=OP.add)
                mn = wpool.tile([P, F_HID], dt.float32, tag="mn")
                nc.vector.tensor_scalar_min(mn[:], h2[:], 0.0)
                nc.scalar.activation(mn[:], mn[:], AF.Exp)
                nc.vector.scalar_tensor_tensor(
                    out=h2[:], in0=h2[:], scalar=0.0, in1=mn[:],
                    op0=OP.max, op1=OP.add)
                nc.vector.tensor_scalar_add(h2[:], h2[:], -1.0)
                pst = ppool.tile([F_HID, P], dt.float32, tag="psT")
                nc.tensor.transpose(out=pst[:], in_=h2[:], identity=ident[:])
                h2T = wpool.tile([F_HID, P], dt.float16, tag="h2T")
                nc.scalar.copy(h2T[:], pst[:])
                ps2 = ppool.tile([P, 42], dt.float32, tag="ps2")
                nc.tensor.matmul(out=ps2[:], lhsT=h2T[:], rhs=w2_t[:],
                                 start=True, stop=True)
                row2 = wpool.tile([P, T2W], dt.float16, tag="t2row")
                nc.scalar.copy(row2[:], ps2[:, 0:T2W])
                nc.sync.dma_start(out=t2_shard[t * P:(t + 1) * P, 0:T2W],
                                  in_=row2[:])
                nc.vector.tensor_copy(ald2[:, t:t + 1], ps2[:, T2W:42])
            dummy2 = wpool.tile([P, PITCH], dt.float16, tag="dummy")
            nc.vector.memset(dummy2[:], BIG_NEG)
            for a, b in runs:
                nc.sync.dma_start(out=t2_shard[a:b, :], in_=dummy2[:b - a, :])

            for b in range(NBANDS):
                po, bs = int(band_pos_off[b]), BSIZES[b]
                to = int(band_tab_off[b])
                nc.gpsimd.collective_compute(
                    "AllGather", OP.bypass,
                    replica_groups=[list(range(CORES))],
                    ins=[t2_shard[po:po + bs, :].opt()],
                    outs=[t2_full[to:to + 8 * bs, :].opt()])

            # ---- phase C2: layer-2 aggregation + log_softmax ----
            mi = 0
            for t in range(TILES):
                NSL = int(SL[t]) + 1
                G2 = gpool.tile([P, NSL * T2W], dt.float16, tag="G2", bufs=5)
                while mi < len(metas) and metas[mi][0] == t:
                    _, b, crel, w, so, NI = metas[mi]
                    to = int(band_tab_off[b])
                    bs8 = 8 * BSIZES[b]
                    raw_gather(
                        G2[:, crel * T2W:(crel + w) * T2W]
                            .rearrange("p (c e) -> p c e", e=T2W),
                        t2_full[to:to + bs8, 0:T2W],
                        idx_t[:, so:so + w * 8], NI, T2W, b)
                    mi += 1
                nc.sync.dma_start(
                    out=G2[:, (NSL - 1) * T2W:NSL * T2W],
                    in_=t2_shard[t * P:(t + 1) * P, 0:T2W])
                p2 = wpool.tile([P, NSL], dt.float32, tag="p2")
                nc.vector.tensor_scalar(
                    out=p2[:],
                    in0=G2[:].rearrange("p (d w) -> p d w", w=T2W)[:, :, 0:1].squeeze(2),
                    scalar1=ald2[:, t:t + 1], scalar2=None, op0=OP.add)
                nc.vector.scalar_tensor_tensor(
                    out=p2[:], in0=p2[:], scalar=NEG_SLOPE, in1=p2[:],
                    op0=OP.mult, op1=OP.max)
                den2 = wpool.tile([P, 1], dt.float32, tag="den2")
                p216 = wpool.tile([P, NSL], dt.float16, tag="p216")
                nc.scalar.activation(p216[:], p2[:], AF.Exp, accum_out=den2[:])
                rec2 = wpool.tile([P, 1], dt.float32, tag="rec2")
                nc.vector.reciprocal(rec2[:], den2[:])
                G2p = wpool.tile([P, NSL * NUM_CLASSES], dt.float16, tag="G2p")
                G2_h = G2[:].rearrange("p (d w) -> p d w", w=T2W)[:, :, 1:T2W]
                p2_b = p216[:].unsqueeze(2).broadcast_to([P, NSL, NUM_CLASSES])
                nc.vector.tensor_tensor(
                    out=G2p[:].rearrange("p (d w) -> p d w", w=NUM_CLASSES),
                    in0=G2_h, in1=p2_b, op=OP.mult)
                o2 = wpool.tile([P, NUM_CLASSES], dt.float32, tag="o2")
                nc.vector.tensor_reduce(
                    out=o2[:],
                    in_=G2p[:].rearrange("p (d w) -> p w d", w=NUM_CLASSES),
                    axis=AX.X, op=OP.add)
                nc.vector.tensor_scalar(out=o2[:], in0=o2[:], scalar1=rec2[:, 0:1],
                                        scalar2=None, op0=OP.mult)
                nc.vector.tensor_tensor(out=o2[:], in0=o2[:], in1=b2_t[:], op=OP.add)
                mx = wpool.tile([P, 1], dt.float32, tag="mx")
                nc.vector.tensor_reduce(out=mx[:], in_=o2[:], axis=AX.X, op=OP.max)
                nc.vector.tensor_scalar(out=o2[:], in0=o2[:], scalar1=mx[:, 0:1],
                                        scalar2=None, op0=OP.subtract)
                ex = wpool.tile([P, NUM_CLASSES], dt.float32, tag="ex")
                sm = wpool.tile([P, 1], dt.float32, tag="sm")
                nc.scalar.activation(ex[:], o2[:], AF.Exp, accum_out=sm[:])
                lg = wpool.tile([P, 1], dt.float32, tag="lg")
                nc.scalar.activation(lg[:], sm[:], AF.Ln)
                nc.vector.tensor_scalar(out=o2[:], in0=o2[:], scalar1=lg[:, 0:1],
                                        scalar2=None, op0=OP.subtract)
                nc.sync.dma_start(out=out[t * P:(t + 1) * P, :], in_=o2[:])
    nc.compile()
    return nc


def _make_runner(nc, n_cores=CORES):
    """Hold a jitted PJRT executable for repeated invocation."""
    import jax
    from jax.sharding import Mesh, PartitionSpec
    from jax.experimental.shard_map import shard_map
    from concourse import mybir
    from concourse.bass2jax import (_bass_exec_p, install_neuronx_cc_hook,
                                    partition_id_tensor)
    install_neuronx_cc_hook()
    partition_name = nc.partition_id_tensor.name if nc.partition_id_tensor else None
    in_names, out_names, out_avals, zero_outs = [], [], [], []
    for alloc in nc.m.functions[0].allocations:
        if not isinstance(alloc, mybir.MemoryLocationSet):
            continue
        name = alloc.memorylocations[0].name
        if alloc.kind == "ExternalInput":
            if name != partition_name:
                in_names.append(name)
        elif alloc.kind == "ExternalOutput":
            shape = tuple(alloc.tensor_shape)
            dtype = mybir.dt.np(alloc.dtype)
            out_names.append(name)
            out_avals.append(jax.core.ShapedArray(shape, dtype))
            zero_outs.append(np.zeros(shape, dtype))
    n_params = len(in_names)
    all_in = list(in_names) + list(out_names) + ([partition_name] if partition_name else [])

    def _body(*args):
        operands = list(args)
        if partition_name is not None:
            operands.append(partition_id_tensor())
        outs = _bass_exec_p.bind(
            *operands, out_avals=tuple(out_avals), in_names=tuple(all_in),
            out_names=tuple(out_names), lowering_input_output_aliases=(),
            sim_require_finite=False, sim_require_nnan=False, nc=nc)
        return tuple(outs)

    devices = jax.devices()[:n_cores]
    mesh = Mesh(np.asarray(devices), ("core",))
    nio = n_params + len(out_names)
    sharded = jax.jit(
        shard_map(_body, mesh=mesh, in_specs=(PartitionSpec("core"),) * nio,
                  out_specs=(PartitionSpec("core"),) * len(out_names),
                  check_rep=False),
        keep_unused=True)

    def run(in_maps, time_reps=0):
        import time as _t
        concat_in = [np.concatenate([np.asarray(in_maps[c][nm])
                                     for c in range(n_cores)], axis=0)
                     for nm in in_names]
        concat_zero = [np.zeros((n_cores * z.shape[0], *z.shape[1:]), z.dtype)
                       for z in zero_outs]
        dev_in = [jax.device_put(a) for a in concat_in]
        dev_zero = [jax.device_put(a) for a in concat_zero]
        outs = sharded(*dev_in, *dev_zero)
        jax.block_until_ready(outs)
        tmin = None
        if time_reps:
            ts = []
            for _ in range(time_reps):
                t0 = _t.perf_counter()
                outs = sharded(*dev_in, *dev_zero)
                jax.block_until_ready(outs)
                ts.append(_t.perf_counter() - t0)
            tmin = min(ts)
        results = [{nm: np.asarray(outs[i]).reshape(n_cores, *out_avals[i].shape)[c]
                    for i, nm in enumerate(out_names)} for c in range(n_cores)]
        return results, tmin

    run.sharded = sharded
    run.in_names = in_names
    run.out_names = out_names
    run.out_avals = out_avals
    return run


def kernel(x, edge_index, W1, a_src1, a_dst1, b1, W2, a_src2, a_dst2, b2,
           _time_reps=0):
    x = np.asarray(x, dtype=np.float32)
    edge_index = np.asarray(edge_index)
    W1 = np.asarray(W1, dtype=np.float32)
    W2 = np.asarray(W2, dtype=np.float32)
    a_src1 = np.asarray(a_src1, dtype=np.float32)
    a_dst1 = np.asarray(a_dst1, dtype=np.float32)
    a_src2 = np.asarray(a_src2, dtype=np.float32)
    a_dst2 = np.asarray(a_dst2, dtype=np.float32)
    b1 = np.asarray(b1, dtype=np.float32)
    b2 = np.asarray(b2, dtype=np.float32)

    meta = _preprocess_graph(edge_index)
    SH = meta["shard_pad"]

    # fold attention vectors into the projections (host, tiny)
    As = np.zeros((F_HID, HEADS), dtype=np.float32)
    Ad = np.zeros((F_HID, HEADS), dtype=np.float32)
    for h in range(HEADS):
        As[h * HID:(h + 1) * HID, h] = a_src1[h]
        Ad[h * HID:(h + 1) * HID, h] = a_dst1[h]
    w1aug = np.concatenate([W1 @ As, W1, W1 @ Ad], axis=1).astype(np.float16)
    w2aug = np.concatenate([W2 @ a_src2.T, W2, W2 @ a_dst2.T], axis=1).astype(np.float16)
    b1rep = np.broadcast_to(b1[None, :], (P, F_HID)).copy()
    b2rep = np.broadcast_to(b2[None, :], (P, NUM_CLASSES)).copy()

    key = (tuple(meta["Dq"].ravel().tolist()), SH)
    if key not in _PROGRAM_CACHE:
        nc = _build_program(meta)
        _PROGRAM_CACHE[key] = _make_runner(nc)
    run = _PROGRAM_CACHE[key]

    in_maps = []
    node_pos = meta["node_pos"]
    offs = meta["offs"]
    for r in range(CORES):
        own = np.arange(offs[r], offs[r + 1])
        xs = np.zeros((SH, F_IN), dtype=np.float16)
        xs[node_pos[own]] = x[own]
        in_maps.append({
            "xT": np.ascontiguousarray(xs.T),
            "w1aug": w1aug, "w2aug": w2aug,
            "b1rep": b1rep, "b2rep": b2rep,
            "idx16": meta["streams"][r],
        })

    try:
        results, tmin = run(in_maps, time_reps=_time_reps)
    except Exception:
        results, tmin = run(in_maps, time_reps=_time_reps)
    out = np.zeros((N, NUM_CLASSES), dtype=np.float32)
    for r in range(CORES):
        own = np.arange(offs[r], offs[r + 1])
        out[own] = results[r]["out"][node_pos[own]]
    if _time_reps:
        kernel._last_time_s = tmin
    return out


# revision 10
# speedup vs baseline: 1.0387x; 1.0387x over previous
"""2-layer GAT (GATConv x2, PyG-style self-loops) on 8 Trainium2 NeuronCores.

Strategy (graph parallel, nodes+incident-edges partitioned by destination):
- Nodes sharded across 8 cores (12500 each, padded to 12544). Each core
  projects its shard (x @ W1aug with the attention vectors folded in),
  writes per-node table rows [al_src | feat] to DRAM, and AllGathers the
  table so every core holds all source rows (halo).
- The per-edge source-row gather is the bottleneck.  Rows are laid out
  band-major: 4 "bands" of positions across all cores, each band section
  <= 25600 rows so a row index fits int16.  Per 128-destination tile,
  incident edges sit in slot columns grouped by source band; each
  (tile, band) range is fetched by batched InstDMAGatherAnt instructions
  (<=896 indices each, one SWDGE queue per band so descriptor generation
  runs on all four Q7 core pairs concurrently).  This measures ~2.4 ns
  per gathered row vs ~8.6 ns/row for the one-column-per-instruction
  indirect-DMA path.
- Attention softmax + weighted aggregation run as strided DVE/ACT ops over
  the gathered block; max-subtraction is skipped (logits are O(1), and
  exp(e-m)/sum == exp(e)/sum analytically).  Slot padding points at
  per-band dummy rows whose al_src is -60000 so exp() contributes 0.
- AllGathers are chunked per band so layer gathers overlap the collective.
- Weights are tiny and replicated; a_src/a_dst are folded into the
  projections on the host: W_aug = [W @ Asrc | W | W @ Adst].
"""

import numpy as np

# Problem constants (hardcoded per spec)
N = 100000
E = 1600000
F_IN = 512
HID = 8
HEADS = 8
F_HID = HID * HEADS  # 64
NUM_CLASSES = 40
NEG_SLOPE = 0.2
CORES = 8
P = 128
PITCH = 128          # fp16 elements per table row (256B DMA stride)
T1W = HEADS + F_HID  # 72: [al_s(8) | h(64)]
T2W = 1 + NUM_CLASSES  # 41: [al_s2(1) | g(40)]
NBANDS = 4
BSIZES = [3200, 3200, 3200, 2944]     # positions per band (per core)
MAX_NI = 896                          # <=1008 (64-desc SWDGE ring)
BIG_NEG = -60000.0

_PROGRAM_CACHE = {}


def _split_shards(n, cores):
    base = n // cores
    rem = n % cores
    sizes = [base + (1 if r < rem else 0) for r in range(cores)]
    offs = np.concatenate([[0], np.cumsum(sizes)])
    return sizes, offs


def _preprocess_graph(edge_index, n=N, cores=CORES):
    src = edge_index[0].astype(np.int64)
    dst = edge_index[1].astype(np.int64)

    sizes, offs = _split_shards(n, cores)
    shard_pad = int(np.ceil((max(sizes) + 1) / P) * P)  # 12544
    tiles = shard_pad // P                              # 98
    assert sum(BSIZES) == shard_pad

    band_pos_off = np.concatenate([[0], np.cumsum(BSIZES)])   # per-core pos
    band_tab_off = np.concatenate([[0], np.cumsum([8 * b for b in BSIZES])])
    caps = [b - 1 for b in BSIZES]  # last row of each band = dummy

    deg = np.bincount(dst, minlength=n)
    node_core = np.searchsorted(offs[1:], np.arange(n), side="right")

    # pass 1: per-core degree sort -> positions (band assignment)
    node_pos = np.full(n, -1, dtype=np.int64)
    real_pos_template = np.concatenate(
        [np.arange(band_pos_off[b], band_pos_off[b] + caps[b])
         for b in range(NBANDS)])
    for r in range(cores):
        own = np.arange(offs[r], offs[r + 1])
        order = own[np.argsort(-deg[own], kind="stable")]
        node_pos[order] = real_pos_template[:len(order)]

    pos_band = np.searchsorted(band_pos_off[1:], np.arange(shard_pad),
                               side="right")
    src_band = pos_band[node_pos[src]]

    # per-node per-band in-degree (for clustering + slot packing)
    degb = np.zeros((n, NBANDS), dtype=np.int64)
    np.add.at(degb, (dst, src_band), 1)

    # pass 2: within each band, cluster nodes into tiles by band-profile
    # (bucketed lexsort) -- band membership (and hence src_band) unchanged.
    for r in range(cores):
        own_mask_pos = np.full(shard_pad, -1, dtype=np.int64)
        own = np.arange(offs[r], offs[r + 1])
        own_mask_pos[node_pos[own]] = own
        for b in range(NBANDS):
            pos_lo = band_pos_off[b]
            nodes_b = own_mask_pos[pos_lo:pos_lo + caps[b]]
            nodes_b = nodes_b[nodes_b >= 0]
            if len(nodes_b) == 0:
                continue
            d = degb[nodes_b]
            bb = d // 4
            key = np.lexsort((-deg[nodes_b], -bb[:, 3], -bb[:, 2],
                              -bb[:, 1], -bb[:, 0]))
            node_pos[nodes_b[key]] = pos_lo + np.arange(len(nodes_b))

    table_row = np.zeros(n, dtype=np.int64)
    pb = pos_band[node_pos]
    table_row = (band_tab_off[pb] +
                 node_core * np.array(BSIZES)[pb] +
                 (node_pos - band_pos_off[pb]))
    src_band = pos_band[node_pos[src]]  # unchanged by pass 2, recompute anyway

    # per-tile per-band slot counts, shared across cores
    Dq = np.zeros((tiles, NBANDS), dtype=np.int64)
    for r in range(cores):
        m = (dst >= offs[r]) & (dst < offs[r + 1])
        lpos = node_pos[dst[m]]
        sb = src_band[m]
        cnt = np.zeros((shard_pad, NBANDS), dtype=np.int64)
        np.add.at(cnt, (lpos, sb), 1)
        Dq = np.maximum(Dq, cnt.reshape(tiles, P, NBANDS).max(axis=1))
    # every tile keeps >=1 band-0 slot (pad-dst den>0 safety)
    Dq[:, 0] = np.maximum(Dq[:, 0], 1)

    SL = Dq.sum(axis=1)                 # slots per tile (excl. self-loop)
    tile_col_off = np.concatenate([[0], np.cumsum(SL)])
    S_cols = int(SL.sum())

    # fill per-core idx16 arrays [P, S_cols]
    dummy_local = [BSIZES[b] - 1 for b in range(NBANDS)]  # core 0's dummy
    band_col_off = np.concatenate(
        [np.zeros((tiles, 1), np.int64), np.cumsum(Dq, axis=1)], axis=1)
    idx_arrays = []
    for r in range(cores):
        idx = np.zeros((P, S_cols), dtype=np.int16)
        for b in range(NBANDS):
            cols_b = []
            for t in range(tiles):
                c0 = tile_col_off[t] + band_col_off[t][b]
                cols_b.append(np.arange(c0, c0 + Dq[t][b]))
            cols_b = np.concatenate(cols_b)
            idx[:, cols_b] = dummy_local[b]
        m = (dst >= offs[r]) & (dst < offs[r + 1])
        e_src = src[m]
        lpos = node_pos[dst[m]]
        sb = src_band[m]
        o = np.lexsort((sb, lpos))
        e_src, lpos, sb = e_src[o], lpos[o], sb[o]
        key = lpos * NBANDS + sb
        grp_start = np.searchsorted(key, np.arange(shard_pad * NBANDS),
                                    side="left")
        slot = np.arange(len(lpos)) - grp_start[key]
        t_of = lpos // P
        part = lpos % P
        col = (tile_col_off[t_of] + band_col_off[t_of, sb] + slot)
        loc = (table_row[e_src] - band_tab_off[sb]).astype(np.int16)
        idx[part, col] = loc
        # pad dst rows: give band-0 slot 0 a real row (local 0)
        npos = np.zeros(shard_pad, dtype=bool)
        own = np.arange(offs[r], offs[r + 1])
        npos[node_pos[own]] = True
        pl = np.nonzero(~npos)[0]
        c0 = tile_col_off[pl // P] + band_col_off[pl // P, 0]
        idx[pl % P, c0] = 0
        idx_arrays.append(idx)

    # gather-instruction metas + int16 streams (wrapped + 8x replicated)
    metas = []  # (tile, band, col_rel, ncols, stream_off16, NI)
    blocks = []
    w_off = 0
    for t in range(tiles):
        chunk_list = []
        for b in range(NBANDS):
            nb = int(Dq[t][b])
            ca = 0
            while ca < nb:
                w = min(MAX_NI // P, nb - ca)
                chunk_list.append((b, ca, w))
                ca += w
        # round-robin across bands so consecutive instrs hit different queues
        chunk_list.sort(key=lambda x: (x[1] // max(x[2], 1), x[0]))
        for b, ca, w in chunk_list:
            NI = w * P
            col0 = int(tile_col_off[t] + band_col_off[t][b] + ca)
            metas.append((t, b, int(band_col_off[t][b] + ca), w, w_off, NI))
            blocks.append((col0, w))
            w_off += w * 8

    def build_stream(idx):
        stream = np.zeros((P, w_off), dtype=np.int16)
        o = 0
        for col0, w in blocks:
            NI = w * P
            flat = idx[:, col0:col0 + w].T.ravel()
            wr = np.zeros((16, w * 8), dtype=np.int16)
            wr[np.arange(NI) % 16, np.arange(NI) // 16] = flat
            stream[:, o:o + w * 8] = np.tile(wr, (8, 1))
            o += w * 8
        return stream

    streams = [build_stream(idx_arrays[r]) for r in range(cores)]

    # pad region: positions never assigned to a real node (incl. band dummies)
    assigned = np.zeros(shard_pad, dtype=bool)
    assigned[node_pos[np.arange(offs[0], offs[1])]] = True  # same per core
    neg_rows = np.nonzero(~assigned)[0]

    return {
        "streams": streams, "metas": metas, "Dq": Dq, "SL": SL,
        "tile_col_off": tile_col_off, "band_col_off": band_col_off,
        "node_pos": node_pos, "sizes": sizes, "offs": offs,
        "shard_pad": shard_pad, "tiles": tiles, "S_cols": S_cols,
        "W16": w_off, "neg_rows": neg_rows,
        "band_pos_off": band_pos_off, "band_tab_off": band_tab_off,
    }


def _build_program(meta):
    from concourse import mybir, bacc
    import concourse.tile as tile
    import concourse.ap_utils as ap_utils
    from concourse.masks import make_identity

    dt = mybir.dt
    SH = meta["shard_pad"]
    TILES = meta["tiles"]
    SL = meta["SL"]
    W16 = meta["W16"]
    NROWS = SH * CORES
    W1C = F_IN // P
    band_pos_off = meta["band_pos_off"]
    band_tab_off = meta["band_tab_off"]
    neg_rows = meta["neg_rows"]
    metas = meta["metas"]

    nc = bacc.Bacc("TRN2", target_bir_lowering=False, debug=False,
                   num_devices=CORES, num_swdge_queues=4)
    xT = nc.dram_tensor("xT", [F_IN, SH], dt.float16, kind="ExternalInput")
    w1aug = nc.dram_tensor("w1aug", [F_IN, 80], dt.float16, kind="ExternalInput")
    w2aug = nc.dram_tensor("w2aug", [F_HID, 42], dt.float16, kind="ExternalInput")
    b1rep = nc.dram_tensor("b1rep", [P, F_HID], dt.float32, kind="ExternalInput")
    b2rep = nc.dram_tensor("b2rep", [P, NUM_CLASSES], dt.float32, kind="ExternalInput")
    idx_in = nc.dram_tensor("idx16", [P, W16], dt.int16, kind="ExternalInput")
    out = nc.dram_tensor("out", [SH, NUM_CLASSES], dt.float32, kind="ExternalOutput")

    AF = mybir.ActivationFunctionType
    OP = mybir.AluOpType
    AX = mybir.AxisListType

    def raw_gather(G_ap, in_ap, idxs_ap, NI, elem, queue):
        g = nc.gpsimd
        assert idxs_ap.dtype == dt.int16
        assert ap_utils.ap_is_contiguous(in_ap.ap[1:])
        assert ap_utils.ap_is_contiguous(G_ap.ap[1:])
        assert ap_utils.ap_is_contiguous(idxs_ap.ap[1:])
        assert in_ap.ap[0][0] == PITCH
        stride_bytes_256 = (PITCH * 2) // 256
        _in_ap = g.lower_ap_dma(in_ap, for_custom_bir_dma=True)
        _idxs_ap = g.lower_ap(idxs_ap)
        _out_ap = g.lower_ap(G_ap)
        return g.add_instruction(
            mybir.InstDMAGatherAnt(
                name=nc.get_next_instruction_name(),
                ins=[*_in_ap, _idxs_ap, g.lower_val_access(g.to_reg(NI))],
                outs=[_out_ap],
                transpose=False, num_idxs=NI, elem_size=elem,
                stride_bytes_256=stride_bytes_256, gen_mode=0,
                single_packet=True, queue_num=queue,
                sbuf_tokens_per_rank=0, sbuf_free_dim_per_rank=0,
                sbuf_free_dim_pad_per_rank=0, sbuf_byte_offset=0))

    with tile.TileContext(nc) as tc:
        with (
            tc.tile_pool(name="const", bufs=1) as cpool,
            tc.tile_pool(name="resident", bufs=1) as rpool,
            tc.tile_pool(name="work", bufs=3) as wpool,
            tc.tile_pool(name="gbuf", bufs=3) as gpool,
            tc.tile_pool(name="psum", bufs=2, space="PSUM") as ppool,
            tc.tile_pool(name="dram", bufs=1, space="DRAM") as dpool,
        ):
            # ---- constants / residents ----
            w1_t = cpool.tile([P, W1C * 80], dt.float16)
            for c in range(W1C):
                nc.sync.dma_start(out=w1_t[:, c * 80:(c + 1) * 80],
                                  in_=w1aug[c * P:(c + 1) * P, :])
            w2_t = cpool.tile([F_HID, 42], dt.float16)
            nc.sync.dma_start(out=w2_t[:], in_=w2aug[:, :])
            b1_t = cpool.tile([P, F_HID], dt.float32)
            nc.sync.dma_start(out=b1_t[:], in_=b1rep[:, :])
            b2_t = cpool.tile([P, NUM_CLASSES], dt.float32)
            nc.sync.dma_start(out=b2_t[:], in_=b2rep[:, :])
            ident = cpool.tile([P, P], dt.float32)
            make_identity(nc, ident[:])
            idx_t = rpool.tile([P, W16], dt.int16)
            nc.sync.dma_start(out=idx_t[:], in_=idx_in[:, :])
            ald1 = rpool.tile([P, TILES * HEADS], dt.float32)
            ald2 = rpool.tile([P, TILES], dt.float32)

            t1_shard = dpool.tile([SH, PITCH], dt.float16)
            t1_full = dpool.tile([NROWS, PITCH], dt.float16)
            t2_shard = dpool.tile([SH, PITCH], dt.float16)
            t2_full = dpool.tile([NROWS, PITCH], dt.float16)

            # ---- phase A: h1 = x @ W1aug per 128-node tile ----
            for t in range(TILES):
                ps = ppool.tile([P, 80], dt.float32, tag="psA")
                lhs = wpool.tile([P, W1C * P], dt.float16, tag="xT")
                nc.sync.dma_start(
                    out=lhs[:].rearrange("p (c n) -> p c n", n=P),
                    in_=xT[:, t * P:(t + 1) * P]
                        .rearrange("(c p) n -> p c n", p=P))
                for c in range(W1C):
                    nc.tensor.matmul(
                        out=ps[:], lhsT=lhs[:, c * P:(c + 1) * P],
                        rhs=w1_t[:, c * 80:(c + 1) * 80],
                        start=(c == 0), stop=(c == W1C - 1))
                row = wpool.tile([P, T1W], dt.float16, tag="t1row")
                nc.scalar.copy(row[:], ps[:, 0:T1W])
                nc.sync.dma_start(out=t1_shard[t * P:(t + 1) * P, 0:T1W],
                                  in_=row[:])
                nc.vector.tensor_copy(ald1[:, t * HEADS:(t + 1) * HEADS],
                                      ps[:, T1W:80])
            # dummy/pad rows -> BIG_NEG so their slots contribute 0
            dummy = wpool.tile([P, PITCH], dt.float16, tag="dummy")
            nc.vector.memset(dummy[:], BIG_NEG)
            runs = []  # contiguous runs of neg_rows
            for rrow in neg_rows:
                if runs and runs[-1][1] == rrow:
                    runs[-1][1] = rrow + 1
                else:
                    runs.append([rrow, rrow + 1])
            for a, b in runs:
                nc.sync.dma_start(out=t1_shard[a:b, :], in_=dummy[:b - a, :])

            # ---- AllGather layer-1 table, chunked per band ----
            for b in range(NBANDS):
                po, bs = int(band_pos_off[b]), BSIZES[b]
                to = int(band_tab_off[b])
                nc.gpsimd.collective_compute(
                    "AllGather", OP.bypass,
                    replica_groups=[list(range(CORES))],
                    ins=[t1_shard[po:po + bs, :].opt()],
                    outs=[t1_full[to:to + 8 * bs, :].opt()])

            # ---- phase C1: layer-1 aggregation + layer-2 projection ----
            mi = 0
            for t in range(TILES):
                NSL = int(SL[t]) + 1  # + self-loop column (last)
                G = gpool.tile([P, NSL * T1W], dt.float16, tag="G1", bufs=6)
                while mi < len(metas) and metas[mi][0] == t:
                    _, b, crel, w, so, NI = metas[mi]
                    to = int(band_tab_off[b])
                    bs8 = 8 * BSIZES[b]
                    raw_gather(
                        G[:, crel * T1W:(crel + w) * T1W]
                            .rearrange("p (c e) -> p c e", e=T1W),
                        t1_full[to:to + bs8, 0:T1W],
                        idx_t[:, so:so + w * 8], NI, T1W, b)
                    mi += 1
                nc.sync.dma_start(
                    out=G[:, (NSL - 1) * T1W:NSL * T1W],
                    in_=t1_shard[t * P:(t + 1) * P, 0:T1W])
                plog = wpool.tile([P, NSL * HEADS], dt.float32, tag="plog")
                G_al = G[:].rearrange("p (d w) -> p d w", w=T1W)[:, :, 0:HEADS]
                ald_b = ald1[:, t * HEADS:(t + 1) * HEADS].unsqueeze(1) \
                    .broadcast_to([P, NSL, HEADS])
                nc.vector.tensor_tensor(
                    out=plog[:].rearrange("p (d w) -> p d w", w=HEADS),
                    in0=G_al, in1=ald_b, op=OP.add)
                nc.vector.scalar_tensor_tensor(
                    out=plog[:], in0=plog[:], scalar=NEG_SLOPE, in1=plog[:],
                    op0=OP.mult, op1=OP.max)
                plog16 = wpool.tile([P, NSL * HEADS], dt.float16, tag="plog16")
                nc.scalar.activation(plog16[:], plog[:], AF.Exp)
                Gp = wpool.tile([P, NSL * F_HID], dt.float16, tag="Gp", bufs=2)
                G_h = G[:].rearrange("p (d w) -> p d w", w=T1W)[:, :, HEADS:T1W] \
                    .rearrange("p d (h f) -> p d h f", f=HID)
                p_b = plog16[:].rearrange("p (d h) -> p d h", h=HEADS) \
                    .unsqueeze(3).broadcast_to([P, NSL, HEADS, HID])
                nc.vector.tensor_tensor(
                    out=Gp[:].rearrange("p (d h f) -> p d h f", h=HEADS, f=HID),
                    in0=G_h, in1=p_b, op=OP.mult)
                den = wpool.tile([P, HEADS], dt.float32, tag="den")
                nc.vector.tensor_reduce(
                    out=den[:],
                    in_=plog16[:].rearrange("p (d w) -> p w d", w=HEADS),
                    axis=AX.X, op=OP.add)
                rec = wpool.tile([P, HEADS], dt.float32, tag="rec")
                nc.vector.reciprocal(rec[:], den[:])
                acc = wpool.tile([P, F_HID], dt.float32, tag="acc")
                nc.vector.tensor_reduce(
                    out=acc[:],
                    in_=Gp[:].rearrange("p (d w) -> p w d", w=F_HID),
                    axis=AX.X, op=OP.add)
                h2 = wpool.tile([P, F_HID], dt.float32, tag="h2")
                rec_b = rec[:].unsqueeze(2).broadcast_to([P, HEADS, HID])
                nc.vector.tensor_tensor(
                    out=h2[:].rearrange("p (h f) -> p h f", f=HID),
                    in0=acc[:].rearrange("p (h f) -> p h f", f=HID),
                    in1=rec_b, op=OP.mult)
                nc.vector.tensor_tensor(out=h2[:], in0=h2[:], in1=b1_t[:], op=OP.add)
                mn = wpool.tile([P, F_HID], dt.float32, tag="mn")
                nc.vector.tensor_scalar_min(mn[:], h2[:], 0.0)
                nc.scalar.activation(mn[:], mn[:], AF.Exp)
                nc.vector.scalar_tensor_tensor(
                    out=h2[:], in0=h2[:], scalar=0.0, in1=mn[:],
                    op0=OP.max, op1=OP.add)
                nc.vector.tensor_scalar_add(h2[:], h2[:], -1.0)
                pst = ppool.tile([F_HID, P], dt.float32, tag="psT")
                nc.tensor.transpose(out=pst[:], in_=h2[:], identity=ident[:])
                h2T = wpool.tile([F_HID, P], dt.float16, tag="h2T")
                nc.scalar.copy(h2T[:], pst[:])
                ps2 = ppool.tile([P, 42], dt.float32, tag="ps2")
                nc.tensor.matmul(out=ps2[:], lhsT=h2T[:], rhs=w2_t[:],
                                 start=True, stop=True)
                row2 = wpool.tile([P, T2W], dt.float16, tag="t2row")
                nc.scalar.copy(row2[:], ps2[:, 0:T2W])
                nc.sync.dma_start(out=t2_shard[t * P:(t + 1) * P, 0:T2W],
                                  in_=row2[:])
                nc.vector.tensor_copy(ald2[:, t:t + 1], ps2[:, T2W:42])
            dummy2 = wpool.tile([P, PITCH], dt.float16, tag="dummy")
            nc.vector.memset(dummy2[:], BIG_NEG)
            for a, b in runs:
                nc.sync.dma_start(out=t2_shard[a:b, :], in_=dummy2[:b - a, :])

            for b in range(NBANDS):
                po, bs = int(band_pos_off[b]), BSIZES[b]
                to = int(band_tab_off[b])
                nc.gpsimd.collective_compute(
                    "AllGather", OP.bypass,
                    replica_groups=[list(range(CORES))],
                    ins=[t2_shard[po:po + bs, :].opt()],
                    outs=[t2_full[to:to + 8 * bs, :].opt()])

            # ---- phase C2: layer-2 aggregation + log_softmax ----
            mi = 0
            for t in range(TILES):
                NSL = int(SL[t]) + 1
                G2 = gpool.tile([P, NSL * T2W], dt.float16, tag="G2", bufs=6)
                while mi < len(metas) and metas[mi][0] == t:
                    _, b, crel, w, so, NI = metas[mi]
                    to = int(band_tab_off[b])
                    bs8 = 8 * BSIZES[b]
                    raw_gather(
                        G2[:, crel * T2W:(crel + w) * T2W]
                            .rearrange("p (c e) -> p c e", e=T2W),
                        t2_full[to:to + bs8, 0:T2W],
                        idx_t[:, so:so + w * 8], NI, T2W, b)
                    mi += 1
                nc.sync.dma_start(
                    out=G2[:, (NSL - 1) * T2W:NSL * T2W],
                    in_=t2_shard[t * P:(t + 1) * P, 0:T2W])
                p2 = wpool.tile([P, NSL], dt.float32, tag="p2")
                nc.vector.tensor_scalar(
                    out=p2[:],
                    in0=G2[:].rearrange("p (d w) -> p d w", w=T2W)[:, :, 0:1].squeeze(2),
                    scalar1=ald2[:, t:t + 1], scalar2=None, op0=OP.add)
                nc.vector.scalar_tensor_tensor(
                    out=p2[:], in0=p2[:], scalar=NEG_SLOPE, in1=p2[:],
                    op0=OP.mult, op1=OP.max)
                den2 = wpool.tile([P, 1], dt.float32, tag="den2")
                p216 = wpool.tile([P, NSL], dt.float16, tag="p216")
                nc.scalar.activation(p216[:], p2[:], AF.Exp, accum_out=den2[:])
                G2p = wpool.tile([P, NSL * NUM_CLASSES], dt.float16, tag="G2p", bufs=2)
                G2_h = G2[:].rearrange("p (d w) -> p d w", w=T2W)[:, :, 1:T2W]
                p2_b = p216[:].unsqueeze(2).broadcast_to([P, NSL, NUM_CLASSES])
                nc.vector.tensor_tensor(
                    out=G2p[:].rearrange("p (d w) -> p d w", w=NUM_CLASSES),
                    in0=G2_h, in1=p2_b, op=OP.mult)
                rec2 = wpool.tile([P, 1], dt.float32, tag="rec2")
                nc.vector.reciprocal(rec2[:], den2[:])
                o2 = wpool.tile([P, NUM_CLASSES], dt.float32, tag="o2")
                nc.vector.tensor_reduce(
                    out=o2[:],
                    in_=G2p[:].rearrange("p (d w) -> p w d", w=NUM_CLASSES),
                    axis=AX.X, op=OP.add)
                nc.vector.tensor_scalar(out=o2[:], in0=o2[:], scalar1=rec2[:, 0:1],
                                        scalar2=None, op0=OP.mult)
                nc.vector.tensor_tensor(out=o2[:], in0=o2[:], in1=b2_t[:], op=OP.add)
                mx = wpool.tile([P, 1], dt.float32, tag="mx")
                nc.vector.tensor_reduce(out=mx[:], in_=o2[:], axis=AX.X, op=OP.max)
                nc.vector.tensor_scalar(out=o2[:], in0=o2[:], scalar1=mx[:, 0:1],
                                        scalar2=None, op0=OP.subtract)
                ex = wpool.tile([P, NUM_CLASSES], dt.float32, tag="ex")
                sm = wpool.tile([P, 1], dt.float32, tag="sm")
                nc.scalar.activation(ex[:], o2[:], AF.Exp, accum_out=sm[:])
                lg = wpool.tile([P, 1], dt.float32, tag="lg")
                nc.scalar.activation(lg[:], sm[:], AF.Ln)
                nc.vector.tensor_scalar(out=o2[:], in0=o2[:], scalar1=lg[:, 0:1],
                                        scalar2=None, op0=OP.subtract)
                nc.sync.dma_start(out=out[t * P:(t + 1) * P, :], in_=o2[:])
    nc.compile()
    return nc


def _make_runner(nc, n_cores=CORES):
    """Hold a jitted PJRT executable for repeated invocation."""
    import jax
    from jax.sharding import Mesh, PartitionSpec
    from jax.experimental.shard_map import shard_map
    from concourse import mybir
    from concourse.bass2jax import (_bass_exec_p, install_neuronx_cc_hook,
                                    partition_id_tensor)
    install_neuronx_cc_hook()
    partition_name = nc.partition_id_tensor.name if nc.partition_id_tensor else None
    in_names, out_names, out_avals, zero_outs = [], [], [], []
    for alloc in nc.m.functions[0].allocations:
        if not isinstance(alloc, mybir.MemoryLocationSet):
            continue
        name = alloc.memorylocations[0].name
        if alloc.kind == "ExternalInput":
            if name != partition_name:
                in_names.append(name)
        elif alloc.kind == "ExternalOutput":
            shape = tuple(alloc.tensor_shape)
            dtype = mybir.dt.np(alloc.dtype)
            out_names.append(name)
            out_avals.append(jax.core.ShapedArray(shape, dtype))
            zero_outs.append(np.zeros(shape, dtype))
    n_params = len(in_names)
    all_in = list(in_names) + list(out_names) + ([partition_name] if partition_name else [])

    def _body(*args):
        operands = list(args)
        if partition_name is not None:
            operands.append(partition_id_tensor())
        outs = _bass_exec_p.bind(
            *operands, out_avals=tuple(out_avals), in_names=tuple(all_in),
            out_names=tuple(out_names), lowering_input_output_aliases=(),
            sim_require_finite=False, sim_require_nnan=False, nc=nc)
        return tuple(outs)

    devices = jax.devices()[:n_cores]
    mesh = Mesh(np.asarray(devices), ("core",))
    nio = n_params + len(out_names)
    sharded = jax.jit(
        shard_map(_body, mesh=mesh, in_specs=(PartitionSpec("core"),) * nio,
                  out_specs=(PartitionSpec("core"),) * len(out_names),
                  check_rep=False),
        keep_unused=True)

    def run(in_maps, time_reps=0):
        import time as _t
        concat_in = [np.concatenate([np.asarray(in_maps[c][nm])
                                     for c in range(n_cores)], axis=0)
                     for nm in in_names]
        concat_zero = [np.zeros((n_cores * z.shape[0], *z.shape[1:]), z.dtype)
                       for z in zero_outs]
        dev_in = [jax.device_put(a) for a in concat_in]
        dev_zero = [jax.device_put(a) for a in concat_zero]
        outs = sharded(*dev_in, *dev_zero)
        jax.block_until_ready(outs)
        tmin = None
        if time_reps:
            ts = []
            for _ in range(time_reps):
                t0 = _t.perf_counter()
                outs = sharded(*dev_in, *dev_zero)
                jax.block_until_ready(outs)
                ts.append(_t.perf_counter() - t0)
            tmin = min(ts)
        results = [{nm: np.asarray(outs[i]).reshape(n_cores, *out_avals[i].shape)[c]
                    for i, nm in enumerate(out_names)} for c in range(n_cores)]
        return results, tmin

    run.sharded = sharded
    run.in_names = in_names
    run.out_names = out_names
    run.out_avals = out_avals
    return run


def kernel(x, edge_index, W1, a_src1, a_dst1, b1, W2, a_src2, a_dst2, b2,
           _time_reps=0):
    x = np.asarray(x, dtype=np.float32)
    edge_index = np.asarray(edge_index)
    W1 = np.asarray(W1, dtype=np.float32)
    W2 = np.asarray(W2, dtype=np.float32)
    a_src1 = np.asarray(a_src1, dtype=np.float32)
    a_dst1 = np.asarray(a_dst1, dtype=np.float32)
    a_src2 = np.asarray(a_src2, dtype=np.float32)
    a_dst2 = np.asarray(a_dst2, dtype=np.float32)
    b1 = np.asarray(b1, dtype=np.float32)
    b2 = np.asarray(b2, dtype=np.float32)

    meta = _preprocess_graph(edge_index)
    SH = meta["shard_pad"]

    # fold attention vectors into the projections (host, tiny)
    As = np.zeros((F_HID, HEADS), dtype=np.float32)
    Ad = np.zeros((F_HID, HEADS), dtype=np.float32)
    for h in range(HEADS):
        As[h * HID:(h + 1) * HID, h] = a_src1[h]
        Ad[h * HID:(h + 1) * HID, h] = a_dst1[h]
    w1aug = np.concatenate([W1 @ As, W1, W1 @ Ad], axis=1).astype(np.float16)
    w2aug = np.concatenate([W2 @ a_src2.T, W2, W2 @ a_dst2.T], axis=1).astype(np.float16)
    b1rep = np.broadcast_to(b1[None, :], (P, F_HID)).copy()
    b2rep = np.broadcast_to(b2[None, :], (P, NUM_CLASSES)).copy()

    key = (tuple(meta["Dq"].ravel().tolist()), SH)
    if key not in _PROGRAM_CACHE:
        nc = _build_program(meta)
        _PROGRAM_CACHE[key] = _make_runner(nc)
    run = _PROGRAM_CACHE[key]

    in_maps = []
    node_pos = meta["node_pos"]
    offs = meta["offs"]
    for r in range(CORES):
        own = np.arange(offs[r], offs[r + 1])
        xs = np.zeros((SH, F_IN), dtype=np.float16)
        xs[node_pos[own]] = x[own]
        in_maps.append({
            "xT": np.ascontiguousarray(xs.T),
            "w1aug": w1aug, "w2aug": w2aug,
            "b1rep": b1rep, "b2rep": b2rep,
            "idx16": meta["streams"][r],
        })

    try:
        results, tmin = run(in_maps, time_reps=_time_reps)
    except Exception:
        results, tmin = run(in_maps, time_reps=_time_reps)
    out = np.zeros((N, NUM_CLASSES), dtype=np.float32)
    for r in range(CORES):
        own = np.arange(offs[r], offs[r + 1])
        out[own] = results[r]["out"][node_pos[own]]
    if _time_reps:
        kernel._last_time_s = tmin
    return out
